# revision 1
# baseline (speedup 1.0000x reference)
"""Trainium2 Bass kernel for nn_Attention_78554951844258.

Dense 12-head attention block: qkv = x@Wqkv+b; RoPE(q,k); softmax(q k^T/sqrt(d)) v; proj.

Sharding: data-parallel over batch — each of the 8 NeuronCores computes one
batch element end-to-end (no collectives).

Algebraic restructuring (host-side, exact, O(weights)):
  * The reference applies RoPE with seq_dim=1 on [b,h,n,d], so cos/sin depend
    only on (head, dim) — RoPE is a position-independent per-head 64x64 linear
    map M_h that folds into the q/k columns of w_qkv (and biases).
  * The softmax scale 1/sqrt(d) folds into the q weights.
  * The v bias and proj bias fold into a single output bias
    b_out = b_v @ w_proj + b_proj, because softmax rows sum to 1.
  * Softmax max-subtraction is skipped: folded scores are bounded (|S| < ~3),
    exp is safe in fp32 and the result is mathematically identical.

Device layout per core (batch element b):
  qk^T [1536, 1024] = (w_qk)^T-stationary matmuls against x^T  (c on partitions)
  v    [1024, 768]  with a ones column appended per head ("v_aug", [j, 12*65])
  S^T  [j, i] per head = k^T-stationary x q^T-moving (K=64, two heads row-packed)
  P^T  = exp(S^T) via ACT;  [out^T | colsums] = [V|1]^T-stationary x P^T-moving
  normalize out^T columns by 1/colsums (DVE recip, broadcast across
  partitions via a DRAM round-trip DMA, DVE multiply; odd head moved into its
  ovT partition range by an SBUF->SBUF DMA)
  y [i, 768] = out^T-stationary x w_proj-moving, + bias via DVE, DMA out.
Matmul operands are bf16 (weights/activations rounded on host or at the
producing engine); accumulation is fp32 in PSUM. Measured ~346 us on HW,
rel l2 err ~1.9e-3 (an fp32r variant measured 400 us at 1.2e-4).
"""
import numpy as np

NUM_HEADS = 12
E = 768
D = 64
B = 8
N = 1024
HALF = D // 2


def _ensure_axon_hooks():
    """The NTFF profile hook registry module may be missing in a fresh
    container; (re)create it so trace=True profiling degrades gracefully."""
    try:
        import antenv.axon_hooks  # noqa: F401
        return
    except ImportError:
        pass
    try:
        import antenv
        import os
        p = os.path.join(os.path.dirname(antenv.__file__), "axon_hooks.py")
        with open(p, "w") as f:
            f.write(
                "_hook = None\n\n"
                "def set_axon_ntff_profile_hook(hook):\n"
                "    global _hook\n    _hook = hook\n\n"
                "def get_axon_ntff_profile_hook():\n"
                "    return _hook\n")
    except Exception:
        pass


_ensure_axon_hooks()


# ---------------------------------------------------------------- host math
def _rope_matrix():
    """M[h, x, d]: rope(q)[x] = sum_d M[h, x, d] * q[d] (float64)."""
    inv_freq = 1.0 / (10000.0 ** (np.arange(0, D, 2, dtype=np.float64) / D))
    t = np.arange(NUM_HEADS, dtype=np.float64)
    emb = np.concatenate([t[:, None] * inv_freq[None, :]] * 2, axis=-1)  # [H, D]
    cos, sin = np.cos(emb), np.sin(emb)
    M = np.zeros((NUM_HEADS, D, D))
    for h in range(NUM_HEADS):
        for d in range(D):
            M[h, d, d] = cos[h, d]
            if d < HALF:
                M[h, d, d + HALF] = -sin[h, d]
            else:
                M[h, d, d - HALF] = sin[h, d]
    return M


def _prep_weights(w_qkv, b_qkv, w_proj, b_proj):
    w = w_qkv.astype(np.float64)
    b = b_qkv.astype(np.float64)
    M = _rope_matrix()
    scale = float(D) ** (-0.5)
    w_q = w[:, 0:E].reshape(E, NUM_HEADS, D)
    w_k = w[:, E:2 * E].reshape(E, NUM_HEADS, D)
    b_q = b[0:E].reshape(NUM_HEADS, D)
    b_k = b[E:2 * E].reshape(NUM_HEADS, D)
    w_q2 = np.einsum('ehd,hxd->ehx', w_q, M) * scale
    b_q2 = np.einsum('hd,hxd->hx', b_q, M) * scale
    w_k2 = np.einsum('ehd,hxd->ehx', w_k, M)
    b_k2 = np.einsum('hd,hxd->hx', b_k, M)
    w_qk = np.ascontiguousarray(
        np.concatenate([w_q2.reshape(E, E), w_k2.reshape(E, E)], axis=1),
        dtype=np.float32)                                     # [E, 2E]
    b_qk = np.concatenate([b_q2.reshape(E), b_k2.reshape(E)]).astype(np.float32)
    w_v = np.ascontiguousarray(w[:, 2 * E:3 * E], dtype=np.float32)
    b_out = (b[2 * E:3 * E] @ w_proj.astype(np.float64)
             + b_proj.astype(np.float64)).astype(np.float32)
    return w_qk, b_qk, w_v, b_out


# ---------------------------------------------------------------- waitfix
def _split_excess_waits(nc):
    """walrus in this container rejects >4 sync waits per instruction (and
    fewer on Drain/SP-NoOp paths). Split overflow waits onto preceding
    same-engine 1-wait NOPs — semantically identical (sequencer blocks in
    order)."""
    import concourse.mybir as mybir
    import bass_rust
    counter = [0]

    def make_nop(engine):
        counter[0] += 1
        nop = bass_rust.InstNoOp(name=f"I-waitfix-{counter[0]}", ins=[], outs=[])
        nop.engine = engine
        return nop

    for fn in nc.m.functions:
        for bb in fn.blocks:
            insts = bb.instructions
            out = []
            changed = False
            for inst in insts:
                si = inst.sync_info
                waits = list(si.on_wait) if si is not None else []
                tn = type(inst).__name__
                # Per-struct wait-slot capacity varies and several structs
                # (S3_LW matmul, S3D3_TS, SP NoOp) reject even small counts;
                # keep at most one wait everywhere, none on Drain (it gets
                # codegen-generated queue waits of its own).
                keep = 0 if tn == "InstDrain" else 1
                if len(waits) > keep:
                    for w in waits[:len(waits) - keep]:
                        nop = make_nop(inst.engine)
                        nop.sync_info = mybir.SyncInfo(on_wait=[w], on_update=[])
                        out.append(nop)
                    inst.sync_info = mybir.SyncInfo(
                        on_wait=waits[len(waits) - keep:],
                        on_update=list(si.on_update))
                    changed = True
                out.append(inst)
            if changed:
                bb.instructions = out


# ---------------------------------------------------------------- device IR
_NC_CACHE = []


def _build_nc():
    import concourse.bass as bass
    import concourse.mybir as mybir
    from concourse.tile import TileContext

    dt = mybir.dt
    f32 = dt.float32
    f32r = dt.float32r
    bf16 = dt.bfloat16
    AF = mybir.ActivationFunctionType

    nc = bass.Bass(target_bir_lowering=False)
    xT_d = nc.dram_tensor("xT", [E, N], bf16, kind="ExternalInput")
    wqk_d = nc.dram_tensor("w_qk", [E, 2 * E], bf16, kind="ExternalInput")
    bqk_d = nc.dram_tensor("b_qk", [2 * E], f32, kind="ExternalInput")
    wv_d = nc.dram_tensor("w_v", [E, E], bf16, kind="ExternalInput")
    wp_d = nc.dram_tensor("w_proj", [E, E], bf16, kind="ExternalInput")
    bo_d = nc.dram_tensor("b_out", [E], f32, kind="ExternalInput")
    y_d = nc.dram_tensor("y", [N, E], f32, kind="ExternalOutput")

    ET = E // 128          # 6 e-tiles
    IT = N // 128          # 8 i/j-tiles
    HP = NUM_HEADS // 2    # 6 head pairs

    with TileContext(nc) as tc:
        with (
            tc.tile_pool(name="phase1", bufs=1) as p1,       # xT, w_qk
            tc.tile_pool(name="persist", bufs=1) as pp,      # v_aug, w_proj, biases
            tc.tile_pool(name="psum", bufs=4, space="PSUM") as ps,
        ):
            # ---- loads
            xT = [p1.tile([128, N], bf16, tag=f"xT{e}", name=f"xT{e}")
                  for e in range(ET)]
            wqk = [p1.tile([128, 2 * E], bf16, tag=f"wqk{e}", name=f"wqk{e}")
                   for e in range(ET)]
            wp = [pp.tile([128, E], bf16, tag=f"wp{e}", name=f"wp{e}")
                  for e in range(ET)]
            for e in range(ET):
                nc.sync.dma_start(out=xT[e], in_=xT_d[e * 128:(e + 1) * 128, :])
                nc.sync.dma_start(out=wqk[e], in_=wqk_d[e * 128:(e + 1) * 128, :])
                nc.sync.dma_start(out=wp[e], in_=wp_d[e * 128:(e + 1) * 128, :])
            bq = pp.tile([128, 12], f32, tag="bq")
            nc.sync.dma_start(out=bq, in_=bqk_d[:].rearrange("(t p) -> p t", p=128))
            bo = pp.tile([128, E], f32, tag="bo")
            nc.sync.dma_start(
                out=bo,
                in_=bass.AP(tensor=bo_d[:].tensor, offset=bo_d[:].offset,
                            ap=[[0, 128], [1, E]]))

            v_aug = [pp.tile([128, NUM_HEADS * (D + 1)], bf16, tag=f"vaug{i}",
                             name=f"vaug{i}") for i in range(IT)]

            # ---- phase 1b: v = x @ w_v, written per-head with ones columns
            with tc.tile_pool(name="pwv", bufs=1) as pwv:
                wv = [pwv.tile([128, E], bf16, tag=f"wv{e}", name=f"wv{e}")
                      for e in range(ET)]
                for e in range(ET):
                    nc.sync.dma_start(out=wv[e],
                                      in_=wv_d[e * 128:(e + 1) * 128, :])
                for it in range(IT):
                    pv = ps.tile([128, E], f32, tag="ps", name=f"pv_{it}")
                    for (n0, nw) in ((0, 512), (512, 256)):
                        for e in range(ET):
                            nc.tensor.matmul(
                                pv[:, n0:n0 + nw],
                                xT[e][:, it * 128:(it + 1) * 128],
                                wv[e][:, n0:n0 + nw],
                                start=(e == 0), stop=(e == ET - 1))
                    for h in range(NUM_HEADS):
                        nc.vector.tensor_copy(
                            out=v_aug[it][:, h * 65:h * 65 + 64],
                            in_=pv[:, h * 64:(h + 1) * 64])
                    # exact 1.0 into the per-head ones columns; memset on a
                    # strided f32r view fails this walrus's ISA check, so use
                    # DVE (in0*0 + 1) with the finite bias tile as dummy input
                    ones_cols = v_aug[it].rearrange(
                        "p (h c) -> p h c", c=65)[:, :, 64:65]
                    bq12 = bq[:, 0:12].rearrange("p (a b) -> p a b", b=1)
                    nc.vector.tensor_scalar(
                        ones_cols, bq12, 0.0, 1.0,
                        mybir.AluOpType.mult, mybir.AluOpType.add)

            with (
                tc.tile_pool(name="pqk", bufs=3) as pqk,     # rotating q^T/k^T
                tc.tile_pool(name="pT", bufs=4) as ppT,      # exp'd scores
                tc.tile_pool(name="late", bufs=1) as pl,     # ovT, rc
                tc.tile_pool(name="nrm", bufs=6) as prb,      # recip broadcast
                tc.tile_pool(name="yout", bufs=2) as py,     # y staging
                tc.tile_pool(name="dscr", bufs=4, space="DRAM") as pdram,
            ):
                ovT = [pl.tile([128, N], bf16, tag=f"ovT{e}", name=f"ovT{e}")
                       for e in range(ET)]

                # ---- phase 1a interleaved with phase 2, one head pair at a time
                for hp in range(HP):
                    h0, h1 = 2 * hp, 2 * hp + 1
                    qt = pqk.tile([128, N], bf16, tag="qkT", name=f"qT{hp}")
                    kt = pqk.tile([128, N], bf16, tag="qkT", name=f"kT{hp}")
                    for ct, dst in ((hp, qt), (ET + hp, kt)):
                        pq = ps.tile([128, N], f32, tag="ps", name=f"pq_{ct}")
                        for ih in range(2):
                            for e in range(ET):
                                nc.tensor.matmul(
                                    pq[:, ih * 512:(ih + 1) * 512],
                                    wqk[e][:, ct * 128:(ct + 1) * 128],
                                    xT[e][:, ih * 512:(ih + 1) * 512],
                                    start=(e == 0), stop=(e == ET - 1))
                        nc.vector.tensor_scalar_add(dst, pq, bq[:, ct:ct + 1])

                    pv0 = ps.tile([65, N], f32, tag="ps", name=f"pv0_{hp}")
                    pv1 = ps.tile([65, N], f32, tag="ps", name=f"pv1_{hp}")
                    for jt in range(IT):
                        js = slice(jt * 128, (jt + 1) * 128)
                        st0 = ps.tile([128, N], f32, tag="ps", name=f"st0_{hp}_{jt}")
                        st1 = ps.tile([128, N], f32, tag="ps", name=f"st1_{hp}_{jt}")
                        for ih in range(2):
                            isl = slice(ih * 512, (ih + 1) * 512)
                            nc.tensor.matmul(st0[:, isl], kt[0:64, js],
                                             qt[0:64, isl])
                            nc.tensor.matmul(st1[:, isl], kt[64:128, js],
                                             qt[64:128, isl])
                        pT0 = ppT.tile([128, N], bf16, tag="pT", name=f"pT0_{hp}_{jt}")
                        pT1 = ppT.tile([128, N], bf16, tag="pT", name=f"pT1_{hp}_{jt}")
                        nc.scalar.activation(out=pT0, in_=st0, func=AF.Exp)
                        nc.scalar.activation(out=pT1, in_=st1, func=AF.Exp)
                        for ih in range(2):
                            isl = slice(ih * 512, (ih + 1) * 512)
                            nc.tensor.matmul(
                                pv0[:, isl], v_aug[jt][:, h0 * 65:h0 * 65 + 65],
                                pT0[:, isl], start=(jt == 0), stop=(jt == IT - 1))
                            nc.tensor.matmul(
                                pv1[:, isl], v_aug[jt][:, h1 * 65:h1 * 65 + 65],
                                pT1[:, isl], start=(jt == 0), stop=(jt == IT - 1))
                    # normalization: recip of column sums (partition 64,
                    # same-lane), gpsimd-broadcast to partitions 0..63,
                    # multiply; odd head moved into place via SBUF->SBUF DMA
                    # (DVE cannot cross partitions).
                    rcp0 = prb.tile([65, N], f32, tag="nrm", name=f"rcp0_{hp}")
                    rcp1 = prb.tile([65, N], f32, tag="nrm", name=f"rcp1_{hp}")
                    nc.vector.reciprocal(out=rcp0[64:65, :], in_=pv0[64:65, :])
                    nc.vector.reciprocal(out=rcp1[64:65, :], in_=pv1[64:65, :])
                    # broadcast via DRAM round-trip (partition_broadcast's ISA
                    # encoding is rejected by this walrus; SBUF APs cannot
                    # have zero partition step, DRAM APs can)
                    ds0 = pdram.tile([1, N], f32, tag="ds", name=f"ds0_{hp}")
                    ds1 = pdram.tile([1, N], f32, tag="ds", name=f"ds1_{hp}")
                    nc.sync.dma_start(out=ds0, in_=rcp0[64:65, :])
                    nc.sync.dma_start(out=ds1, in_=rcp1[64:65, :])
                    rb0 = prb.tile([64, N], f32, tag="nrm", name=f"rb0_{hp}")
                    rb1 = prb.tile([64, N], f32, tag="nrm", name=f"rb1_{hp}")
                    nc.sync.dma_start(
                        out=rb0, in_=bass.AP(tensor=ds0.tensor, offset=ds0.offset,
                                             ap=[[0, 64], [1, N]]))
                    nc.sync.dma_start(
                        out=rb1, in_=bass.AP(tensor=ds1.tensor, offset=ds1.offset,
                                             ap=[[0, 64], [1, N]]))
                    nc.vector.tensor_mul(ovT[hp][0:64, :], pv0[0:64, :], rb0)
                    tmp1 = prb.tile([64, N], bf16, tag="nrm", name=f"tmp1_{hp}")
                    nc.vector.tensor_mul(tmp1, pv1[0:64, :], rb1)
                    nc.sync.dma_start(out=ovT[hp][64:128, :], in_=tmp1)

                # ---- phase 3: y = ovT^T @ w_proj + b_out
                for it in range(IT):
                    isl = slice(it * 128, (it + 1) * 128)
                    pyt = ps.tile([128, E], f32, tag="ps", name=f"py_{it}")
                    for (n0, nw) in ((0, 512), (512, 256)):
                        for e in range(ET):
                            nc.tensor.matmul(
                                pyt[:, n0:n0 + nw],
                                ovT[e][:, isl],
                                wp[e][:, n0:n0 + nw],
                                start=(e == 0), stop=(e == ET - 1))
                    ysb = py.tile([128, E], f32, tag="y", name=f"y{it}")
                    nc.vector.tensor_add(ysb, pyt, bo)
                    nc.sync.dma_start(out=y_d[isl, :], in_=ysb)

    _split_excess_waits(nc)
    return nc


def _get_nc():
    if not _NC_CACHE:
        _NC_CACHE.append(_build_nc())
    return _NC_CACHE[0]


# ---------------------------------------------------------------- entry point
def kernel(x, w_qkv, b_qkv, w_proj, b_proj, _trace=False):
    from concourse.bass_utils import run_bass_kernel_spmd

    import ml_dtypes
    bf16 = ml_dtypes.bfloat16
    x = np.asarray(x)
    w_qk, b_qk, w_v, b_out = _prep_weights(
        np.asarray(w_qkv), np.asarray(b_qkv), np.asarray(w_proj),
        np.asarray(b_proj))
    w_qk16 = w_qk.astype(bf16)
    w_v16 = w_v.astype(bf16)
    w_proj16 = np.ascontiguousarray(np.asarray(w_proj)).astype(bf16)

    in_maps = []
    for b in range(B):
        in_maps.append({
            "xT": np.ascontiguousarray(x[b].T).astype(bf16),
            "w_qk": w_qk16,
            "b_qk": b_qk,
            "w_v": w_v16,
            "w_proj": w_proj16,
            "b_out": b_out,
        })

    nc = _get_nc()
    res = run_bass_kernel_spmd(nc, in_maps, core_ids=list(range(B)),
                               trace=_trace)
    out = np.stack([res.results[b]["y"] for b in range(B)]).astype(np.float32)
    if _trace:
        return out, res
    return out



# revision 6
# speedup vs baseline: 1.0300x; 1.0300x over previous
"""Trainium2 Bass kernel for nn_Attention_78554951844258.

Dense 12-head attention block: qkv = x@Wqkv+b; RoPE(q,k); softmax(q k^T/sqrt(d)) v; proj.

Sharding: data-parallel over batch — each of the 8 NeuronCores computes one
batch element end-to-end (no collectives).

Algebraic restructuring (host-side, exact, O(weights)):
  * The reference applies RoPE with seq_dim=1 on [b,h,n,d], so cos/sin depend
    only on (head, dim) — RoPE is a position-independent per-head 64x64 linear
    map M_h that folds into the q/k columns of w_qkv (and biases).
  * The softmax scale 1/sqrt(d) folds into the q weights.
  * The v bias and proj bias fold into a single output bias
    b_out = b_v @ w_proj + b_proj, because softmax rows sum to 1.
  * Softmax max-subtraction is skipped: folded scores are bounded (|S| < ~3),
    exp is safe in fp32 and the result is mathematically identical.

v2 schedule (HAM-aware): the v1 kernel ran ~360us because each head-pair's
normalization tail (6.5us single-partition DVE reciprocals feeding the next
pair's bias-adds through the in-order DVE queue) idled the PE >3.4us, HAM
re-throttled to 1.2GHz, and ~70% of the matmul stream ran at half clock.
v2 keeps the PE dense:
  * input DMAs ordered by consumption (xT, wv, wqk, biases, wp);
  * v_aug and the first q/k column-tiles pipeline through one 2-buffer PSUM
    tag before attention;
  * the remaining 10 q/k projections stream as PE filler inside the
    attention loop (their own 2-bank PSUM slot), hidden under the ACT-bound
    exp pipeline;
  * PSUM = pq(2 banks) + st(2) + pv(2x2) = 8 banks exactly;
  * normalization uses reciprocal_approx_fast (~1.3us, 18-bit) and gates
    nothing but the pair's own ovT; broadcast via DRAM round-trip DMA;
  * proj accumulates e=0..4 ahead of the last pair's e=5 chunks to shrink
    the tail.
Matmul operands are bf16; accumulation fp32 in PSUM.
"""
import numpy as np

NUM_HEADS = 12
E = 768
D = 64
B = 8
N = 1024
HALF = D // 2


def _ensure_axon_hooks():
    """The NTFF profile hook registry module may be missing in a fresh
    container; (re)create it so trace=True profiling degrades gracefully."""
    try:
        import antenv.axon_hooks  # noqa: F401
        return
    except ImportError:
        pass
    try:
        import antenv
        import os
        p = os.path.join(os.path.dirname(antenv.__file__), "axon_hooks.py")
        with open(p, "w") as f:
            f.write(
                "_hook = None\n\n"
                "def set_axon_ntff_profile_hook(hook):\n"
                "    global _hook\n    _hook = hook\n\n"
                "def get_axon_ntff_profile_hook():\n"
                "    return _hook\n")
    except Exception:
        pass


_ensure_axon_hooks()


# ---------------------------------------------------------------- host math
def _rope_matrix():
    """M[h, x, d]: rope(q)[x] = sum_d M[h, x, d] * q[d] (float64)."""
    inv_freq = 1.0 / (10000.0 ** (np.arange(0, D, 2, dtype=np.float64) / D))
    t = np.arange(NUM_HEADS, dtype=np.float64)
    emb = np.concatenate([t[:, None] * inv_freq[None, :]] * 2, axis=-1)  # [H, D]
    cos, sin = np.cos(emb), np.sin(emb)
    M = np.zeros((NUM_HEADS, D, D))
    for h in range(NUM_HEADS):
        for d in range(D):
            M[h, d, d] = cos[h, d]
            if d < HALF:
                M[h, d, d + HALF] = -sin[h, d]
            else:
                M[h, d, d - HALF] = sin[h, d]
    return M


def _prep_weights(w_qkv, b_qkv, w_proj, b_proj):
    w = w_qkv.astype(np.float64)
    b = b_qkv.astype(np.float64)
    M = _rope_matrix()
    scale = float(D) ** (-0.5)
    w_q = w[:, 0:E].reshape(E, NUM_HEADS, D)
    w_k = w[:, E:2 * E].reshape(E, NUM_HEADS, D)
    b_q = b[0:E].reshape(NUM_HEADS, D)
    b_k = b[E:2 * E].reshape(NUM_HEADS, D)
    w_q2 = np.einsum('ehd,hxd->ehx', w_q, M) * scale
    b_q2 = np.einsum('hd,hxd->hx', b_q, M) * scale
    w_k2 = np.einsum('ehd,hxd->ehx', w_k, M)
    b_k2 = np.einsum('hd,hxd->hx', b_k, M)
    w_qk = np.ascontiguousarray(
        np.concatenate([w_q2.reshape(E, E), w_k2.reshape(E, E)], axis=1),
        dtype=np.float32)                                     # [E, 2E]
    b_qk = np.concatenate([b_q2.reshape(E), b_k2.reshape(E)]).astype(np.float32)
    w_v = np.ascontiguousarray(w[:, 2 * E:3 * E], dtype=np.float32)
    b_out = (b[2 * E:3 * E] @ w_proj.astype(np.float64)
             + b_proj.astype(np.float64)).astype(np.float32)
    return w_qk, b_qk, w_v, b_out


# ---------------------------------------------------------------- waitfix
def _split_excess_waits(nc):
    """walrus in this container rejects >4 sync waits per instruction (and
    fewer on Drain/SP-NoOp paths). Split overflow waits onto preceding
    same-engine 1-wait NOPs — semantically identical (sequencer blocks in
    order)."""
    import concourse.mybir as mybir
    import bass_rust
    counter = [0]

    def make_nop(engine):
        counter[0] += 1
        nop = bass_rust.InstNoOp(name=f"I-waitfix-{counter[0]}", ins=[], outs=[])
        nop.engine = engine
        return nop

    for fn in nc.m.functions:
        for bb in fn.blocks:
            insts = bb.instructions
            out = []
            changed = False
            for inst in insts:
                si = inst.sync_info
                waits = list(si.on_wait) if si is not None else []
                tn = type(inst).__name__
                keep = 0 if tn == "InstDrain" else 1
                if len(waits) > keep:
                    for w in waits[:len(waits) - keep]:
                        nop = make_nop(inst.engine)
                        nop.sync_info = mybir.SyncInfo(on_wait=[w], on_update=[])
                        out.append(nop)
                    inst.sync_info = mybir.SyncInfo(
                        on_wait=waits[len(waits) - keep:],
                        on_update=list(si.on_update))
                    changed = True
                out.append(inst)
            if changed:
                bb.instructions = out


# ---------------------------------------------------------------- device IR
_NC_CACHE = []


def _build_nc():
    import concourse.bass as bass
    import concourse.mybir as mybir
    from concourse.tile import TileContext

    dt = mybir.dt
    f32 = dt.float32
    bf16 = dt.bfloat16
    AF = mybir.ActivationFunctionType

    nc = bass.Bass(target_bir_lowering=False)
    xT_d = nc.dram_tensor("xT", [E, N], bf16, kind="ExternalInput")
    wqk_d = nc.dram_tensor("w_qk", [E, 2 * E], bf16, kind="ExternalInput")
    bqk_d = nc.dram_tensor("b_qk", [2 * E], f32, kind="ExternalInput")
    wv_d = nc.dram_tensor("w_v", [E, E], bf16, kind="ExternalInput")
    wp_d = nc.dram_tensor("w_proj", [E, E], bf16, kind="ExternalInput")
    bo_d = nc.dram_tensor("b_out", [E], f32, kind="ExternalInput")
    y_d = nc.dram_tensor("y", [N, E], f32, kind="ExternalOutput")

    ET = E // 128          # 6 e-tiles
    IT = N // 128          # 8 i/j-tiles
    HP = NUM_HEADS // 2    # 6 head pairs

    with TileContext(nc) as tc:
        with (
            tc.tile_pool(name="stat", bufs=1) as p1,         # xT, w_qk, wv, wp
            tc.tile_pool(name="persist", bufs=1) as pp,      # v_aug, qkt, ovT, biases
            tc.tile_pool(name="pT", bufs=4) as ppT,          # exp'd scores
            tc.tile_pool(name="nrm", bufs=6) as prb,         # recip + broadcast
            tc.tile_pool(name="yout", bufs=2) as py,         # y staging
            tc.tile_pool(name="dscr", bufs=4, space="DRAM") as pdram,
        ):
            # ---- loads, ordered by first use
            xT = [p1.tile([128, N], bf16, tag=f"xT{e}", name=f"xT{e}")
                  for e in range(ET)]
            wv = [p1.tile([128, E], bf16, tag=f"wv{e}", name=f"wv{e}")
                  for e in range(ET)]
            wqk = [p1.tile([128, 2 * E], bf16, tag=f"wqk{e}", name=f"wqk{e}")
                   for e in range(ET)]
            wp = [p1.tile([128, E], bf16, tag=f"wp{e}", name=f"wp{e}")
                  for e in range(ET)]
            for e in range(ET):
                nc.sync.dma_start(out=xT[e], in_=xT_d[e * 128:(e + 1) * 128, :])
            for e in range(ET):
                nc.sync.dma_start(out=wv[e], in_=wv_d[e * 128:(e + 1) * 128, :])
            for e in range(ET):
                nc.sync.dma_start(out=wqk[e], in_=wqk_d[e * 128:(e + 1) * 128, :])
            bq = pp.tile([128, 12], f32, tag="bq")
            nc.sync.dma_start(out=bq, in_=bqk_d[:].rearrange("(t p) -> p t", p=128))
            bo = pp.tile([128, E], f32, tag="bo")
            nc.sync.dma_start(
                out=bo,
                in_=bass.AP(tensor=bo_d[:].tensor, offset=bo_d[:].offset,
                            ap=[[0, 128], [1, E]]))
            for e in range(ET):
                nc.sync.dma_start(out=wp[e], in_=wp_d[e * 128:(e + 1) * 128, :])

            v_aug = [pp.tile([128, NUM_HEADS * (D + 1)], bf16, tag=f"vaug{i}",
                             name=f"vaug{i}") for i in range(IT)]
            qkt = [pp.tile([128, N], bf16, tag=f"qkt{c}", name=f"qkt{c}")
                   for c in range(2 * ET)]

            # q/k column-tile projection: 12 accumulating MMs + bias-add.
            # Emitted eagerly for ct 0/6, then as PE filler inside attention.
            def emit_qk_mm(pq, ct, i):
                ih, e = divmod(i, ET)
                nc.tensor.matmul(
                    pq[:, ih * 512:(ih + 1) * 512],
                    wqk[e][:, ct * 128:(ct + 1) * 128],
                    xT[e][:, ih * 512:(ih + 1) * 512],
                    start=(e == 0), stop=(e == ET - 1))

            def emit_qk_bias(pq, ct):
                nc.vector.tensor_scalar_add(qkt[ct], pq, bq[:, ct:ct + 1])

            # ---- pre-attention: v_aug (8 tiles) + q/k tiles ct=0,6 through
            # one 2-buffer psum tag (scoped pool; space reclaimed after)
            with tc.tile_pool(name="psA", bufs=2, space="PSUM") as psA:
                for it in range(IT):
                    pvv = psA.tile([128, N], f32, tag="vq", name=f"pv_{it}")
                    for (n0, nw) in ((0, 512), (512, 256)):
                        for e in range(ET):
                            nc.tensor.matmul(
                                pvv[:, n0:n0 + nw],
                                xT[e][:, it * 128:(it + 1) * 128],
                                wv[e][:, n0:n0 + nw],
                                start=(e == 0), stop=(e == ET - 1))
                    for h in range(NUM_HEADS):
                        nc.vector.tensor_copy(
                            out=v_aug[it][:, h * 65:h * 65 + 64],
                            in_=pvv[:, h * 64:(h + 1) * 64])
                    # exact 1.0 into the per-head ones columns (DVE in0*0 + 1;
                    # strided memset is rejected by this walrus's ISA check)
                    ones_cols = v_aug[it].rearrange(
                        "p (h c) -> p h c", c=65)[:, :, 64:65]
                    bq12 = bq[:, 0:12].rearrange("p (a b) -> p a b", b=1)
                    nc.vector.tensor_scalar(
                        ones_cols, bq12, 0.0, 1.0,
                        mybir.AluOpType.mult, mybir.AluOpType.add)
                for ct in (0, ET):
                    pq = psA.tile([128, N], f32, tag="vq", name=f"pq_{ct}")
                    for i in range(12):
                        emit_qk_mm(pq, ct, i)
                    emit_qk_bias(pq, ct)

            # ---- attention: per head-pair; next pair's q/k projections
            # interleave as PE filler (3 MMs per jt slot)
            ovT = [pp.tile([128, N], bf16, tag=f"ovT{e}", name=f"ovT{e}")
                   for e in range(ET)]

            with tc.tile_pool(name="psB", bufs=1, space="PSUM") as psB:
                for hp in range(HP):
                    h0, h1 = 2 * hp, 2 * hp + 1
                    qt, kt = qkt[hp], qkt[ET + hp]
                    # filler stream: projections for pair hp+1 (cts hp+1, 7+hp)
                    filler = []
                    if hp + 1 < HP:
                        filler = [(hp + 1, i) for i in range(12)] + \
                                 [(ET + hp + 1, i) for i in range(12)]
                    fpq = [None, None]  # psum tiles for the two filler cts

                    def pop_filler(k):
                        for _ in range(k):
                            if not filler:
                                return
                            ct, i = filler.pop(0)
                            half = 0 if ct < ET else 1
                            if i == 0:
                                fpq[half] = psB.tile([128, N], f32, tag="pq",
                                                     bufs=1, name=f"fpq_{ct}")
                            emit_qk_mm(fpq[half], ct, i)
                            if i == 11:
                                emit_qk_bias(fpq[half], ct)

                    pv0 = psB.tile([65, N], f32, tag="pv", bufs=2,
                                   name=f"pv0_{hp}")
                    pv1 = psB.tile([65, N], f32, tag="pv", bufs=2,
                                   name=f"pv1_{hp}")
                    for jt in range(IT):
                        js = slice(jt * 128, (jt + 1) * 128)
                        st0 = psB.tile([128, N], f32, tag="st", bufs=1,
                                       name=f"st0_{hp}_{jt}")
                        for ih in range(2):
                            isl = slice(ih * 512, (ih + 1) * 512)
                            nc.tensor.matmul(st0[:, isl], kt[0:64, js],
                                             qt[0:64, isl])
                        pT0 = ppT.tile([128, N], bf16, tag="pT",
                                       name=f"pT0_{hp}_{jt}")
                        nc.scalar.activation(out=pT0, in_=st0, func=AF.Exp)
                        pop_filler(2)
                        st1 = psB.tile([128, N], f32, tag="st", bufs=1,
                                       name=f"st1_{hp}_{jt}")
                        for ih in range(2):
                            isl = slice(ih * 512, (ih + 1) * 512)
                            nc.tensor.matmul(st1[:, isl], kt[64:128, js],
                                             qt[64:128, isl])
                        pT1 = ppT.tile([128, N], bf16, tag="pT",
                                       name=f"pT1_{hp}_{jt}")
                        nc.scalar.activation(out=pT1, in_=st1, func=AF.Exp)
                        pop_filler(1)
                        for ih in range(2):
                            isl = slice(ih * 512, (ih + 1) * 512)
                            nc.tensor.matmul(
                                pv0[:, isl],
                                v_aug[jt][:, h0 * 65:h0 * 65 + 65],
                                pT0[:, isl], start=(jt == 0),
                                stop=(jt == IT - 1))
                        for ih in range(2):
                            isl = slice(ih * 512, (ih + 1) * 512)
                            nc.tensor.matmul(
                                pv1[:, isl],
                                v_aug[jt][:, h1 * 65:h1 * 65 + 65],
                                pT1[:, isl], start=(jt == 0),
                                stop=(jt == IT - 1))
                    pop_filler(24)  # drain any remainder

                    # normalization: copy the two colsum rows (PSUM partition
                    # 64) to SBUF, round-trip through DRAM reshaped to
                    # [128,16] so the multi-cycle DVE reciprocal runs on 128
                    # lanes (~0.3us, not 6.5us), then DRAM-broadcast 1/s to
                    # 64 partitions (SBUF APs cannot have zero partition
                    # step, DRAM APs can) and multiply. Odd head moved into
                    # its ovT partition range by DMA (DVE cannot cross
                    # partitions).
                    s0 = prb.tile([65, N], f32, tag="nrm", name=f"s0_{hp}")
                    s1 = prb.tile([65, N], f32, tag="nrm", name=f"s1_{hp}")
                    nc.vector.tensor_copy(out=s0[64:65, :], in_=pv0[64:65, :])
                    nc.vector.tensor_copy(out=s1[64:65, :], in_=pv1[64:65, :])
                    ds = pdram.tile([2, N], f32, tag="ds", name=f"ds_{hp}")
                    nc.sync.dma_start(out=ds[0:1, :], in_=s0[64:65, :])
                    nc.sync.dma_start(out=ds[1:2, :], in_=s1[64:65, :])
                    rsq = prb.tile([128, 16], f32, tag="rsq", name=f"rsq_{hp}")
                    nc.sync.dma_start(
                        out=rsq,
                        in_=bass.AP(tensor=ds.tensor, offset=ds.offset,
                                    ap=[[16, 128], [1, 16]]))
                    rrq = prb.tile([128, 16], f32, tag="rsq", name=f"rrq_{hp}")
                    nc.vector.reciprocal(out=rrq, in_=rsq)
                    dr = pdram.tile([2, N], f32, tag="dr", name=f"dr_{hp}")
                    nc.sync.dma_start(
                        out=bass.AP(tensor=dr.tensor, offset=dr.offset,
                                    ap=[[16, 128], [1, 16]]),
                        in_=rrq)
                    rb0 = prb.tile([64, N], f32, tag="nrm", name=f"rb0_{hp}")
                    rb1 = prb.tile([64, N], f32, tag="nrm", name=f"rb1_{hp}")
                    nc.sync.dma_start(
                        out=rb0,
                        in_=bass.AP(tensor=dr.tensor, offset=dr.offset,
                                    ap=[[0, 64], [1, N]]))
                    nc.sync.dma_start(
                        out=rb1,
                        in_=bass.AP(tensor=dr.tensor, offset=dr[1:2, :].offset,
                                    ap=[[0, 64], [1, N]]))
                    nc.vector.tensor_mul(ovT[hp][0:64, :], pv0[0:64, :], rb0)
                    tmp1 = prb.tile([64, N], bf16, tag="nrm", name=f"tmp1_{hp}")
                    nc.vector.tensor_mul(tmp1, pv1[0:64, :], rb1)
                    nc.sync.dma_start(out=ovT[hp][64:128, :], in_=tmp1)

            # ---- proj: y = ovT^T @ w_proj + b_out. Emit e=0..4 for an
            # it-pair before their e=5 chunks so only the last head-pair's
            # ovT gates the tail.
            with tc.tile_pool(name="psC", bufs=4, space="PSUM") as psC:
                for g in range(IT // 2):
                    its = (2 * g, 2 * g + 1)
                    pyts = {}
                    for it in its:
                        isl = slice(it * 128, (it + 1) * 128)
                        pyt = psC.tile([128, E], f32, tag="py",
                                       name=f"py_{it}")
                        pyts[it] = pyt
                        for (n0, nw) in ((0, 512), (512, 256)):
                            for e in range(ET - 1):
                                nc.tensor.matmul(
                                    pyt[:, n0:n0 + nw],
                                    ovT[e][:, isl],
                                    wp[e][:, n0:n0 + nw],
                                    start=(e == 0), stop=False)
                    for it in its:
                        isl = slice(it * 128, (it + 1) * 128)
                        pyt = pyts[it]
                        for (n0, nw) in ((0, 512), (512, 256)):
                            nc.tensor.matmul(
                                pyt[:, n0:n0 + nw],
                                ovT[ET - 1][:, isl],
                                wp[ET - 1][:, n0:n0 + nw],
                                start=False, stop=True)
                        ysb = py.tile([128, E], f32, tag="y", name=f"y{it}")
                        nc.vector.tensor_add(ysb, pyt, bo)
                        nc.sync.dma_start(out=y_d[isl, :], in_=ysb)

    _split_excess_waits(nc)
    return nc


def _get_nc():
    if not _NC_CACHE:
        _NC_CACHE.append(_build_nc())
    return _NC_CACHE[0]


# ---------------------------------------------------------------- entry point
def kernel(x, w_qkv, b_qkv, w_proj, b_proj, _trace=False):
    from concourse.bass_utils import run_bass_kernel_spmd

    import ml_dtypes
    bf16 = ml_dtypes.bfloat16
    x = np.asarray(x)
    w_qk, b_qk, w_v, b_out = _prep_weights(
        np.asarray(w_qkv), np.asarray(b_qkv), np.asarray(w_proj),
        np.asarray(b_proj))
    w_qk16 = w_qk.astype(bf16)
    w_v16 = w_v.astype(bf16)
    w_proj16 = np.ascontiguousarray(np.asarray(w_proj)).astype(bf16)

    in_maps = []
    for b in range(B):
        in_maps.append({
            "xT": np.ascontiguousarray(x[b].T).astype(bf16),
            "w_qk": w_qk16,
            "b_qk": b_qk,
            "w_v": w_v16,
            "w_proj": w_proj16,
            "b_out": b_out,
        })

    nc = _get_nc()
    res = run_bass_kernel_spmd(nc, in_maps, core_ids=list(range(B)),
                               trace=_trace)
    out = np.stack([res.results[b]["y"] for b in range(B)]).astype(np.float32)
    if _trace:
        return out, res
    return out


# revision 8
# speedup vs baseline: 1.2102x; 1.1749x over previous
"""Trainium2 Bass kernel for nn_Attention_78554951844258.

Dense 12-head attention block: qkv = x@Wqkv+b; RoPE(q,k); softmax(q k^T/sqrt(d)) v; proj.

Sharding: data-parallel over batch — each of the 8 NeuronCores computes one
batch element end-to-end (no collectives).

Algebraic restructuring (host-side, exact, O(weights)):
  * The reference applies RoPE with seq_dim=1 on [b,h,n,d], so cos/sin depend
    only on (head, dim) — RoPE is a position-independent per-head 64x64 linear
    map M_h that folds into the q/k columns of w_qkv (and biases).
  * The softmax scale 1/sqrt(d) folds into the q weights.
  * The v bias and proj bias fold into a single output bias
    b_out = b_v @ w_proj + b_proj, because softmax rows sum to 1.
  * Softmax max-subtraction is skipped: folded scores are bounded (|S| < ~3),
    exp is safe in fp32 and the result is mathematically identical.

v2 schedule (HAM-aware): the v1 kernel ran ~360us because each head-pair's
normalization tail (6.5us single-partition DVE reciprocals feeding the next
pair's bias-adds through the in-order DVE queue) idled the PE >3.4us, HAM
re-throttled to 1.2GHz, and ~70% of the matmul stream ran at half clock.
v2 keeps the PE dense:
  * input DMAs ordered by consumption (xT, wv, wqk, biases, wp);
  * v_aug and the first q/k column-tiles pipeline through one 2-buffer PSUM
    tag before attention;
  * the remaining 10 q/k projections stream as PE filler inside the
    attention loop (their own 2-bank PSUM slot), hidden under the ACT-bound
    exp pipeline;
  * PSUM = pq(2 banks) + st(2) + pv(2x2) = 8 banks exactly;
  * normalization uses reciprocal_approx_fast (~1.3us, 18-bit) and gates
    nothing but the pair's own ovT; broadcast via DRAM round-trip DMA;
  * proj accumulates e=0..4 ahead of the last pair's e=5 chunks to shrink
    the tail.
Matmul operands are bf16; accumulation fp32 in PSUM.
"""
import numpy as np

NUM_HEADS = 12
E = 768
D = 64
B = 8
N = 1024
HALF = D // 2


def _ensure_axon_hooks():
    """The NTFF profile hook registry module may be missing in a fresh
    container; (re)create it so trace=True profiling degrades gracefully."""
    try:
        import antenv.axon_hooks  # noqa: F401
        return
    except ImportError:
        pass
    try:
        import antenv
        import os
        p = os.path.join(os.path.dirname(antenv.__file__), "axon_hooks.py")
        with open(p, "w") as f:
            f.write(
                "_hook = None\n\n"
                "def set_axon_ntff_profile_hook(hook):\n"
                "    global _hook\n    _hook = hook\n\n"
                "def get_axon_ntff_profile_hook():\n"
                "    return _hook\n")
    except Exception:
        pass


_ensure_axon_hooks()


# ---------------------------------------------------------------- host math
def _rope_matrix():
    """M[h, x, d]: rope(q)[x] = sum_d M[h, x, d] * q[d] (float64)."""
    inv_freq = 1.0 / (10000.0 ** (np.arange(0, D, 2, dtype=np.float64) / D))
    t = np.arange(NUM_HEADS, dtype=np.float64)
    emb = np.concatenate([t[:, None] * inv_freq[None, :]] * 2, axis=-1)  # [H, D]
    cos, sin = np.cos(emb), np.sin(emb)
    M = np.zeros((NUM_HEADS, D, D))
    for h in range(NUM_HEADS):
        for d in range(D):
            M[h, d, d] = cos[h, d]
            if d < HALF:
                M[h, d, d + HALF] = -sin[h, d]
            else:
                M[h, d, d - HALF] = sin[h, d]
    return M


def _prep_weights(w_qkv, b_qkv, w_proj, b_proj):
    w = w_qkv.astype(np.float64)
    b = b_qkv.astype(np.float64)
    M = _rope_matrix()
    scale = float(D) ** (-0.5)
    w_q = w[:, 0:E].reshape(E, NUM_HEADS, D)
    w_k = w[:, E:2 * E].reshape(E, NUM_HEADS, D)
    b_q = b[0:E].reshape(NUM_HEADS, D)
    b_k = b[E:2 * E].reshape(NUM_HEADS, D)
    w_q2 = np.einsum('ehd,hxd->ehx', w_q, M) * scale
    b_q2 = np.einsum('hd,hxd->hx', b_q, M) * scale
    w_k2 = np.einsum('ehd,hxd->ehx', w_k, M)
    b_k2 = np.einsum('hd,hxd->hx', b_k, M)
    w_qk = np.ascontiguousarray(
        np.concatenate([w_q2.reshape(E, E), w_k2.reshape(E, E)], axis=1),
        dtype=np.float32)                                     # [E, 2E]
    b_qk = np.concatenate([b_q2.reshape(E), b_k2.reshape(E)]).astype(np.float32)
    w_v = np.ascontiguousarray(w[:, 2 * E:3 * E], dtype=np.float32)
    b_out = (b[2 * E:3 * E] @ w_proj.astype(np.float64)
             + b_proj.astype(np.float64)).astype(np.float32)
    return w_qk, b_qk, w_v, b_out


# ---------------------------------------------------------------- waitfix
def _split_excess_waits(nc):
    """walrus in this container rejects >4 sync waits per instruction (and
    fewer on Drain/SP-NoOp paths). Split overflow waits onto preceding
    same-engine 1-wait NOPs — semantically identical (sequencer blocks in
    order)."""
    import concourse.mybir as mybir
    import bass_rust
    counter = [0]

    def make_nop(engine):
        counter[0] += 1
        nop = bass_rust.InstNoOp(name=f"I-waitfix-{counter[0]}", ins=[], outs=[])
        nop.engine = engine
        return nop

    for fn in nc.m.functions:
        for bb in fn.blocks:
            insts = bb.instructions
            out = []
            changed = False
            for inst in insts:
                si = inst.sync_info
                waits = list(si.on_wait) if si is not None else []
                tn = type(inst).__name__
                keep = 0 if tn == "InstDrain" else 1
                if len(waits) > keep:
                    for w in waits[:len(waits) - keep]:
                        nop = make_nop(inst.engine)
                        nop.sync_info = mybir.SyncInfo(on_wait=[w], on_update=[])
                        out.append(nop)
                    inst.sync_info = mybir.SyncInfo(
                        on_wait=waits[len(waits) - keep:],
                        on_update=list(si.on_update))
                    changed = True
                out.append(inst)
            if changed:
                bb.instructions = out


# ---------------------------------------------------------------- device IR
_NC_CACHE = []


def _build_nc():
    import concourse.bass as bass
    import concourse.mybir as mybir
    from concourse.tile import TileContext

    dt = mybir.dt
    f32 = dt.float32
    bf16 = dt.bfloat16
    AF = mybir.ActivationFunctionType

    nc = bass.Bass(target_bir_lowering=False)
    xT_d = nc.dram_tensor("xT", [E, N], bf16, kind="ExternalInput")
    wqk_d = nc.dram_tensor("w_qk", [E, 2 * E], bf16, kind="ExternalInput")
    bqk_d = nc.dram_tensor("b_qk", [2 * E], f32, kind="ExternalInput")
    wv_d = nc.dram_tensor("w_v", [E, E], bf16, kind="ExternalInput")
    wp_d = nc.dram_tensor("w_proj", [E, E], bf16, kind="ExternalInput")
    bo_d = nc.dram_tensor("b_out", [E], f32, kind="ExternalInput")
    y_d = nc.dram_tensor("y", [N, E], f32, kind="ExternalOutput")

    ET = E // 128          # 6 e-tiles
    IT = N // 128          # 8 i/j-tiles
    HP = NUM_HEADS // 2    # 6 head pairs

    with TileContext(nc) as tc:
        with (
            tc.tile_pool(name="stat", bufs=1) as p1,         # xT, w_qk, wv, wp
            tc.tile_pool(name="persist", bufs=1) as pp,      # v_aug, qkt, ovT, biases
            tc.tile_pool(name="pT", bufs=4) as ppT,          # exp'd scores
            tc.tile_pool(name="nrm", bufs=6) as prb,         # recip + broadcast
            tc.tile_pool(name="yout", bufs=2) as py,         # y staging
            tc.tile_pool(name="dscr", bufs=4, space="DRAM") as pdram,
        ):
            # ---- loads, ordered by first use
            xT = [p1.tile([128, N], bf16, tag=f"xT{e}", name=f"xT{e}")
                  for e in range(ET)]
            wv = [p1.tile([128, E], bf16, tag=f"wv{e}", name=f"wv{e}")
                  for e in range(ET)]
            wqk = [p1.tile([128, 2 * E], bf16, tag=f"wqk{e}", name=f"wqk{e}")
                   for e in range(ET)]
            wp = [p1.tile([128, E], bf16, tag=f"wp{e}", name=f"wp{e}")
                  for e in range(ET)]
            for e in range(ET):
                nc.sync.dma_start(out=xT[e], in_=xT_d[e * 128:(e + 1) * 128, :])
            for e in range(ET):
                nc.sync.dma_start(out=wv[e], in_=wv_d[e * 128:(e + 1) * 128, :])
            for e in range(ET):
                nc.sync.dma_start(out=wqk[e], in_=wqk_d[e * 128:(e + 1) * 128, :])
            bq = pp.tile([128, 12], f32, tag="bq")
            nc.sync.dma_start(out=bq, in_=bqk_d[:].rearrange("(t p) -> p t", p=128))
            bo = pp.tile([128, E], f32, tag="bo")
            nc.sync.dma_start(
                out=bo,
                in_=bass.AP(tensor=bo_d[:].tensor, offset=bo_d[:].offset,
                            ap=[[0, 128], [1, E]]))
            for e in range(ET):
                nc.sync.dma_start(out=wp[e], in_=wp_d[e * 128:(e + 1) * 128, :])

            v_aug = [pp.tile([128, NUM_HEADS * (D + 1)], bf16, tag=f"vaug{i}",
                             name=f"vaug{i}") for i in range(IT)]
            qkt = [pp.tile([128, N], bf16, tag=f"qkt{c}", name=f"qkt{c}")
                   for c in range(2 * ET)]

            # q/k column-tile projection: 12 accumulating MMs + bias-add.
            # Emitted eagerly for ct 0/6, then as PE filler inside attention.
            def emit_qk_mm(pq, ct, i):
                ih, e = divmod(i, ET)
                nc.tensor.matmul(
                    pq[:, ih * 512:(ih + 1) * 512],
                    wqk[e][:, ct * 128:(ct + 1) * 128],
                    xT[e][:, ih * 512:(ih + 1) * 512],
                    start=(e == 0), stop=(e == ET - 1))

            def emit_qk_bias(pq, ct):
                nc.vector.tensor_scalar_add(qkt[ct], pq, bq[:, ct:ct + 1])

            # ---- pre-attention: v_aug (8 tiles) + q/k tiles ct=0,6 through
            # one 2-buffer psum tag (scoped pool; space reclaimed after)
            with tc.tile_pool(name="psA", bufs=2, space="PSUM") as psA:
                for it in range(IT):
                    pvv = psA.tile([128, N], f32, tag="vq", name=f"pv_{it}")
                    for (n0, nw) in ((0, 512), (512, 256)):
                        for e in range(ET):
                            nc.tensor.matmul(
                                pvv[:, n0:n0 + nw],
                                xT[e][:, it * 128:(it + 1) * 128],
                                wv[e][:, n0:n0 + nw],
                                start=(e == 0), stop=(e == ET - 1))
                    for h in range(NUM_HEADS):
                        nc.vector.tensor_copy(
                            out=v_aug[it][:, h * 65:h * 65 + 64],
                            in_=pvv[:, h * 64:(h + 1) * 64])
                    # exact 1.0 into the per-head ones columns (DVE in0*0 + 1;
                    # strided memset is rejected by this walrus's ISA check)
                    ones_cols = v_aug[it].rearrange(
                        "p (h c) -> p h c", c=65)[:, :, 64:65]
                    bq12 = bq[:, 0:12].rearrange("p (a b) -> p a b", b=1)
                    nc.vector.tensor_scalar(
                        ones_cols, bq12, 0.0, 1.0,
                        mybir.AluOpType.mult, mybir.AluOpType.add)
                # all 12 q/k column tiles, interleaved with nothing else —
                # the attention pool needs every PSUM bank, so these cannot
                # hide inside the attention loop
                for ct in list(range(ET)) + list(range(ET, 2 * ET)):
                    pq = psA.tile([128, N], f32, tag="vq", name=f"pq_{ct}")
                    for i in range(12):
                        emit_qk_mm(pq, ct, i)
                    emit_qk_bias(pq, ct)

            # ---- attention: per head-pair; next pair's q/k projections
            # interleave as PE filler (3 MMs per jt slot)
            ovT = [pp.tile([128, N], bf16, tag=f"ovT{e}", name=f"ovT{e}")
                   for e in range(ET)]

            with tc.tile_pool(name="psB", bufs=1, space="PSUM") as psB:
                for hp in range(HP):
                    h0, h1 = 2 * hp, 2 * hp + 1
                    qt, kt = qkt[hp], qkt[ET + hp]
                    pv0 = psB.tile([65, N], f32, tag="pv", bufs=2,
                                   name=f"pv0_{hp}")
                    pv1 = psB.tile([65, N], f32, tag="pv", bufs=2,
                                   name=f"pv1_{hp}")
                    for jt in range(IT):
                        js = slice(jt * 128, (jt + 1) * 128)
                        st0 = psB.tile([128, N], f32, tag="st", bufs=2,
                                       name=f"st0_{hp}_{jt}")
                        for ih in range(2):
                            isl = slice(ih * 512, (ih + 1) * 512)
                            nc.tensor.matmul(st0[:, isl], kt[0:64, js],
                                             qt[0:64, isl])
                        pT0 = ppT.tile([128, N], bf16, tag="pT",
                                       name=f"pT0_{hp}_{jt}")
                        nc.scalar.activation(out=pT0, in_=st0, func=AF.Exp)
                        st1 = psB.tile([128, N], f32, tag="st", bufs=2,
                                       name=f"st1_{hp}_{jt}")
                        for ih in range(2):
                            isl = slice(ih * 512, (ih + 1) * 512)
                            nc.tensor.matmul(st1[:, isl], kt[64:128, js],
                                             qt[64:128, isl])
                        pT1 = ppT.tile([128, N], bf16, tag="pT",
                                       name=f"pT1_{hp}_{jt}")
                        nc.scalar.activation(out=pT1, in_=st1, func=AF.Exp)
                        for ih in range(2):
                            isl = slice(ih * 512, (ih + 1) * 512)
                            nc.tensor.matmul(
                                pv0[:, isl],
                                v_aug[jt][:, h0 * 65:h0 * 65 + 65],
                                pT0[:, isl], start=(jt == 0),
                                stop=(jt == IT - 1))
                        for ih in range(2):
                            isl = slice(ih * 512, (ih + 1) * 512)
                            nc.tensor.matmul(
                                pv1[:, isl],
                                v_aug[jt][:, h1 * 65:h1 * 65 + 65],
                                pT1[:, isl], start=(jt == 0),
                                stop=(jt == IT - 1))

                    # normalization: copy the two colsum rows (PSUM partition
                    # 64) to SBUF, round-trip through DRAM reshaped to
                    # [128,16] so the multi-cycle DVE reciprocal runs on 128
                    # lanes (~0.3us, not 6.5us), then DRAM-broadcast 1/s to
                    # 64 partitions (SBUF APs cannot have zero partition
                    # step, DRAM APs can) and multiply. Odd head moved into
                    # its ovT partition range by DMA (DVE cannot cross
                    # partitions).
                    s0 = prb.tile([65, N], f32, tag="nrm", name=f"s0_{hp}")
                    s1 = prb.tile([65, N], f32, tag="nrm", name=f"s1_{hp}")
                    nc.vector.tensor_copy(out=s0[64:65, :], in_=pv0[64:65, :])
                    nc.vector.tensor_copy(out=s1[64:65, :], in_=pv1[64:65, :])
                    ds = pdram.tile([2, N], f32, tag="ds", name=f"ds_{hp}")
                    nc.sync.dma_start(out=ds[0:1, :], in_=s0[64:65, :])
                    nc.sync.dma_start(out=ds[1:2, :], in_=s1[64:65, :])
                    rsq = prb.tile([128, 16], f32, tag="rsq", name=f"rsq_{hp}")
                    nc.sync.dma_start(
                        out=rsq,
                        in_=bass.AP(tensor=ds.tensor, offset=ds.offset,
                                    ap=[[16, 128], [1, 16]]))
                    rrq = prb.tile([128, 16], f32, tag="rsq", name=f"rrq_{hp}")
                    nc.vector.reciprocal(out=rrq, in_=rsq)
                    dr = pdram.tile([2, N], f32, tag="dr", name=f"dr_{hp}")
                    nc.sync.dma_start(
                        out=bass.AP(tensor=dr.tensor, offset=dr.offset,
                                    ap=[[16, 128], [1, 16]]),
                        in_=rrq)
                    rb0 = prb.tile([64, N], f32, tag="nrm", name=f"rb0_{hp}")
                    rb1 = prb.tile([64, N], f32, tag="nrm", name=f"rb1_{hp}")
                    nc.sync.dma_start(
                        out=rb0,
                        in_=bass.AP(tensor=dr.tensor, offset=dr.offset,
                                    ap=[[0, 64], [1, N]]))
                    nc.sync.dma_start(
                        out=rb1,
                        in_=bass.AP(tensor=dr.tensor, offset=dr[1:2, :].offset,
                                    ap=[[0, 64], [1, N]]))
                    nc.vector.tensor_mul(ovT[hp][0:64, :], pv0[0:64, :], rb0)
                    tmp1 = prb.tile([64, N], bf16, tag="nrm", name=f"tmp1_{hp}")
                    nc.vector.tensor_mul(tmp1, pv1[0:64, :], rb1)
                    nc.sync.dma_start(out=ovT[hp][64:128, :], in_=tmp1)

            # ---- proj: y = ovT^T @ w_proj + b_out. Emit e=0..4 for an
            # it-pair before their e=5 chunks so only the last head-pair's
            # ovT gates the tail.
            with tc.tile_pool(name="psC", bufs=4, space="PSUM") as psC:
                for g in range(IT // 2):
                    its = (2 * g, 2 * g + 1)
                    pyts = {}
                    for it in its:
                        isl = slice(it * 128, (it + 1) * 128)
                        pyt = psC.tile([128, E], f32, tag="py",
                                       name=f"py_{it}")
                        pyts[it] = pyt
                        for (n0, nw) in ((0, 512), (512, 256)):
                            for e in range(ET - 1):
                                nc.tensor.matmul(
                                    pyt[:, n0:n0 + nw],
                                    ovT[e][:, isl],
                                    wp[e][:, n0:n0 + nw],
                                    start=(e == 0), stop=False)
                    for it in its:
                        isl = slice(it * 128, (it + 1) * 128)
                        pyt = pyts[it]
                        for (n0, nw) in ((0, 512), (512, 256)):
                            nc.tensor.matmul(
                                pyt[:, n0:n0 + nw],
                                ovT[ET - 1][:, isl],
                                wp[ET - 1][:, n0:n0 + nw],
                                start=False, stop=True)
                        ysb = py.tile([128, E], f32, tag="y", name=f"y{it}")
                        nc.vector.tensor_add(ysb, pyt, bo)
                        nc.sync.dma_start(out=y_d[isl, :], in_=ysb)

    _split_excess_waits(nc)
    return nc


def _get_nc():
    if not _NC_CACHE:
        _NC_CACHE.append(_build_nc())
    return _NC_CACHE[0]


# ---------------------------------------------------------------- entry point
def kernel(x, w_qkv, b_qkv, w_proj, b_proj, _trace=False):
    from concourse.bass_utils import run_bass_kernel_spmd

    import ml_dtypes
    bf16 = ml_dtypes.bfloat16
    x = np.asarray(x)
    w_qk, b_qk, w_v, b_out = _prep_weights(
        np.asarray(w_qkv), np.asarray(b_qkv), np.asarray(w_proj),
        np.asarray(b_proj))
    w_qk16 = w_qk.astype(bf16)
    w_v16 = w_v.astype(bf16)
    w_proj16 = np.ascontiguousarray(np.asarray(w_proj)).astype(bf16)

    in_maps = []
    for b in range(B):
        in_maps.append({
            "xT": np.ascontiguousarray(x[b].T).astype(bf16),
            "w_qk": w_qk16,
            "b_qk": b_qk,
            "w_v": w_v16,
            "w_proj": w_proj16,
            "b_out": b_out,
        })

    nc = _get_nc()
    res = run_bass_kernel_spmd(nc, in_maps, core_ids=list(range(B)),
                               trace=_trace)
    out = np.stack([res.results[b]["y"] for b in range(B)]).astype(np.float32)
    if _trace:
        return out, res
    return out
